# revision 6
# baseline (speedup 1.0000x reference)
"""Trainium2 Bass kernel for the seasonal-decomposition block.

Math: for each season s, circ_s = real(F_s^H diag(d_s) F_s) with F_s the s-th
diagonal LxL block of the normalized N=L*S DFT matrix. Expanding,
    circ_s[a, b] = (1/N) * sum_j d_s[j] * cos(2*pi*(s*L+j)*(a-b)/N)
depends only on a-b: a symmetric Toeplitz matrix whose first column
c_s(t) is computed on host with one length-N FFT. Every 128x128 block of
circ_s is a contiguous column slice of the skewed buffer
    E2r_s[p, m] = c_s(|2047 + p - m|)   (shape [128, 4096])
so the LxL matrix is never materialized.

The recurrence  x_rem <- x_rem - tanh(x_rem @ circ_s)  runs in transposed
layout (positions on partitions, rows on the free axis). The seasonal
matmuls run in fp8-e4m3 with MatmulPerfMode.DoubleRow: adjacent chunk
pairs (a=2t, 2t+1) occupy adjacent 128-column windows of the skewed
buffer, so a contiguous 256-column slice is a valid [128, 2, 128] lhsT
and one DoubleRow matmul retires two 128-deep accumulation steps at
2 rows/cycle. Weights are pre-scaled by 2^14 on host (max |c|*2^14 ~ 170
< 240) and the tanh activation descales with scale=2^-14. x_rem state
stays fp32 on-chip (fresh fp8 cast each season).

DoubleRow's LDWEIGHTS loads 256 columns (~184 ns) vs ~120 ns of matmul
per instruction, so a chunk-major loop is weight-load-bound. Instead the
loop is WINDOW-major: each distinct weight window (u = b - 2t, the pair
diagonal) is loaded once with an explicit ldweights and then reused by
several no-load matmuls (ldweights=False) accumulating into different
chunks' PSUM tiles. Chunks are processed in two blocks of 8 (PSUM holds
8 live fp32 accumulators + trend), and the season direction alternates
(asc/desc) so each season first consumes the x8 chunks the previous
season produced first - no inter-season PE stall on the tanh/sub/cast
chain. Measured end-to-end rel err ~1.2e-2 < 2e-2 tolerance.

The trailing avg-pool trend runs in fp32r as two banded matmuls per
128-chunk, interleaved at season-3 block boundaries (the PE array is
reloaded at the next explicit ldweights anyway). The output is
accumulated as sum_s tanh_s + trend (never x - x_rem, which would lose
precision to cancellation).

Sharding: pure data-parallel over the B*C = 2048 rows, 256 rows per core,
8 cores, no collectives.
"""

import sys

sys.path.insert(0, "/opt/trn_rl_repo")

import ml_dtypes
import numpy as np

import concourse.mybir as mybir
import concourse.tile as tile
from concourse import bacc
from concourse.bass_utils import run_bass_kernel_spmd

L = 2048
S = 4
NFULL = L * S
KER = 25
B, C = 64, 32
NCORES = 8
ROWS = B * C          # 2048
RPC = ROWS // NCORES  # 256 rows per core
NCHUNK = L // 128     # 16
NPAIR = NCHUNK // 2   # 8

_f32 = mybir.dt.float32
_f32r = mybir.dt.float32r
_f8 = mybir.dt.float8e4
_np_f8 = ml_dtypes.float8_e4m3
SCALE_W = 2.0 ** 14


def _build_tband():
    """Three [128,128] band blocks of the avg-pool matrix T (trend = T.T @ x)."""
    u = np.arange(128)[:, None]
    t = np.arange(128)[None, :]
    diag = ((t - u >= 0) & (t - u <= KER - 1)).astype(np.float32) / KER
    sub = ((u - t) >= 128 - (KER - 1)).astype(np.float32) / KER
    t00 = diag.copy()
    t00[0, :] += np.maximum(0, (KER - 1) - np.arange(128)).astype(np.float32) / KER
    return np.ascontiguousarray(np.stack([t00, diag, sub], axis=1))  # [128, 3, 128]


_TBAND = _build_tband()
_E2R_IDX = np.clip(np.abs(2047 + np.arange(128)[:, None] - np.arange(4096)[None, :]), 0, L - 1)
# chunk order with pairs swapped: position 2t holds chunk 2t+1 and vice versa,
# so pair t's plane0 is chunk 2t+1, matching the weight window layout.
_PAIR_IDX = np.arange(NCHUNK) ^ 1


def _circ_cols(diagonals):
    """First columns c_s(t), t = 0..L-1, of each season's Toeplitz circ_s."""
    d = np.zeros((S, NFULL), dtype=np.float64)
    d[:, :L] = np.asarray(diagonals, dtype=np.float64)
    F = np.fft.fft(d, axis=1)  # F[s,k] = sum_j d_j e^{-2pi i jk/N}
    t = np.arange(L)
    ph = np.exp((2j * np.pi / NFULL) * (np.arange(S)[:, None] * L * t[None, :]))
    return ((ph * np.conj(F[:, :L])).real / NFULL).astype(np.float32)  # [S, L]


def _matmul_noload(nc, out, lhsT, rhs, start, stop, perf_mode):
    """InstMatmult with ldweights=False: use the stationary operand already
    loaded by a preceding explicit nc.tensor.ldweights. Mirrors the tail of
    bass's matmul() lowering; lhsT stays in ins[1] for dependency tracking
    and CoreSim correctness."""
    eng = nc.tensor
    keep_dims = {0, 1}
    ifmap_ap = eng.lower_ap(rhs.opt(keep_dims), opt=False)
    weights_ap = eng.lower_ap(lhsT.opt(keep_dims), opt=False, for_matmul_weights=True)
    out_ap = eng.lower_ap(out)
    return eng.add_instruction(
        mybir.InstMatmult(
            name=eng.bass.get_next_instruction_name(),
            replication_resolution=0,
            replication_shift_amnt=0,
            replication_num_rows=0,
            start_tensor_calc=start,
            stop_tensor_calc=stop,
            ins=[ifmap_ap, weights_ap],
            outs=[out_ap],
            perf_mode=perf_mode,
            is_transpose=None,
            ifmap_quant_offset=None,
            weights_quant_offset=None,
            bass_skip_group_check=False,
            tile_position=(0, 0),
            tile_size=(128, 128),
            ldweights=False,
        )
    )


def _emit_body(nc, pools, xr_d, x8_d, w8_d, tb_d, out_d, noload=True):
    constp, x8p, xrp, corrp, workp, psum_a, psum_t = pools
    tanh_f = mybir.ActivationFunctionType.Tanh
    dr = mybir.MatmulPerfMode.DoubleRow

    # Prologue DMA order follows first use: season-0 matmuls need x8 + w8[0];
    # the fp32 x quarters feed the DVE subs that start ~1us later.
    x8_t0 = x8p.tile([128, NPAIR, 2, RPC], _f8, tag="x8", name="x8_0")
    w8_sb = [constp.tile([128, 4096], _f8, tag=f"w8_{s}", name=f"w8_{s}") for s in range(S)]
    x0t = [constp.tile([128, 4, RPC], _f32r, tag=f"x0_{k}", name=f"x0_{k}") for k in range(4)]
    nc.sync.dma_start(x8_t0[:], x8_d[:])
    nc.sync.dma_start(w8_sb[0][:], w8_d[0])
    for k in range(4):
        nc.sync.dma_start(x0t[k][:], xr_d[k])
    for s in range(1, S):
        nc.sync.dma_start(w8_sb[s][:], w8_d[s])
    tb_sb = constp.tile([128, 3, 128], _f32r, tag="tb")
    nc.sync.dma_start(tb_sb[:], tb_d[:])

    xr_cur = [x0t[a // 4][:, a % 4, :] for a in range(NCHUNK)]
    corr = [corrp.tile([128, RPC], _f32, tag=f"corr{b}", name=f"corr{b}") for b in range(NCHUNK)]
    big_ob = constp.tile([128, NCHUNK, RPC], _f32, tag="bigob")

    def emit_trend(j, xr3):
        tps = psum_t.tile([128, RPC], _f32, tag="acc" if psum_t is psum_a else "tps", name=f"tps{j}")
        if j == 0:
            nc.tensor.matmul(tps[:], tb_sb[:, 0, :], xr3[0], start=True, stop=True)
        else:
            nc.tensor.matmul(tps[:], tb_sb[:, 2, :], xr3[j - 1], start=True, stop=False)
            nc.tensor.matmul(tps[:], tb_sb[:, 1, :], xr3[j], start=False, stop=True)
        nc.vector.tensor_add(out=big_ob[:, j, :], in0=corr[j][:], in1=tps[:])
        if j % 4 == 0:
            q = j // 4
            nc.sync.dma_start(out_d[:, 4 * q : 4 * q + 4, :], big_ob[:, 4 * q : 4 * q + 4, :])

    x8_cur = x8_t0
    for s in range(S):
        asc = (s % 2 == 0)
        xr_next = [xrp.tile([128, RPC], _f32r, tag=f"xr{b}", name=f"xr{s}_{b}") for b in range(NCHUNK)]
        x8_next = (
            x8p.tile([128, NPAIR, 2, RPC], _f8, tag="x8", name=f"x8_{s + 1}")
            if s < S - 1
            else None
        )
        acc = {}

        def finish_chunk(b):
            if s == 0:
                nc.scalar.activation(corr[b][:], acc[b][:], tanh_f, scale=1.0 / SCALE_W)
                tmp_ap = corr[b][:]
            else:
                tmp = workp.tile([128, RPC], _f32, tag="tanh")
                nc.scalar.activation(tmp[:], acc[b][:], tanh_f, scale=1.0 / SCALE_W)
                nc.vector.tensor_add(out=corr[b][:], in0=corr[b][:], in1=tmp[:])
                tmp_ap = tmp[:]
            nc.vector.tensor_sub(out=xr_next[b][:], in0=xr_cur[b], in1=tmp_ap)
            if s < S - 1:
                # fp8 cast for next season's rhs; plane0 = odd chunk, plane1 = even.
                nc.gpsimd.tensor_copy(
                    out=x8_next[:, b // 2, 1 - (b % 2), :], in_=xr_next[b][:]
                )

        blocks = [range(0, 8), range(8, 16)]
        if not asc:
            blocks = blocks[::-1]
        for blk in blocks:
            b0 = blk.start
            us = range(b0 - 14, b0 + 8)
            if not asc:
                us = reversed(us)
            for u in us:
                bs = [b for b in blk if (b - u) % 2 == 0 and 0 <= (b - u) // 2 <= 7]
                if not bs:
                    continue
                col0 = 1919 + 128 * u
                w_ap = w8_sb[s][:, col0 : col0 + 256].rearrange(
                    "p (two f) -> p two f", two=2
                )
                if noload:
                    nc.tensor.ldweights(w_ap, perf_mode=dr)
                for b in bs:
                    t = (b - u) // 2
                    first_u = b - 14 if asc else b
                    last_u = b if asc else b - 14
                    if b not in acc:
                        acc[b] = psum_a.tile([128, RPC], _f32, tag="acc", name=f"acc{s}_{b}")
                    if noload:
                        _matmul_noload(
                            nc, acc[b][:], w_ap, x8_cur[:, t, :, :],
                            start=(u == first_u), stop=(u == last_u), perf_mode=dr,
                        )
                    else:
                        nc.tensor.matmul(
                            acc[b][:], w_ap, x8_cur[:, t, :, :],
                            start=(u == first_u), stop=(u == last_u), perf_mode=dr,
                        )
                    if u == last_u:
                        finish_chunk(b)
            # Season 3 runs desc (blocks 15..8 then 7..0, completing chunks in
            # descending order); trend chunk j needs xr[j] and xr[j-1], so after
            # each block the newly-completed trends interleave here - the PE
            # array is reloaded at the next explicit ldweights anyway.
            if s == S - 1 and b0 == 8:
                for j in (15, 14, 13, 12, 11, 10, 9):
                    emit_trend(j, xr_next)
        xr_cur = [t_[:] for t_ in xr_next]
        x8_cur = x8_next

    for j in (8, 7, 6, 5, 4, 3, 2, 1, 0):
        emit_trend(j, xr_cur)


def build_nc(reps=1, acc_bufs=8, merge_tps=True, noload=True):
    nc = bacc.Bacc("TRN2", target_bir_lowering=False, debug=False)
    xr_d = nc.dram_tensor("xr", [4, 128, 4, RPC], _f32r, kind="ExternalInput")
    x8_d = nc.dram_tensor("x8", [128, NPAIR, 2, RPC], _f8, kind="ExternalInput")
    w8_d = nc.dram_tensor("w8", [S, 128, 4096], _f8, kind="ExternalInput")
    tb_d = nc.dram_tensor("tb", [128, 3, 128], _f32r, kind="ExternalInput")
    out_d = nc.dram_tensor("out", [128, NCHUNK, RPC], _f32, kind="ExternalOutput")

    with tile.TileContext(nc) as tc:
        with (
            tc.tile_pool(name="const", bufs=1) as constp,
            tc.tile_pool(name="x8p", bufs=2) as x8p,
            tc.tile_pool(name="xrp", bufs=2) as xrp,
            tc.tile_pool(name="corrp", bufs=1) as corrp,
            tc.tile_pool(name="work", bufs=4) as workp,
            tc.tile_pool(name="psum_a", bufs=acc_bufs, space="PSUM") as psum_a,
            tc.tile_pool(
                name="psum_t", bufs=(1 if merge_tps else 2), space="PSUM"
            ) as psum_t,
        ):
            pools = (constp, x8p, xrp, corrp, workp, psum_a,
                     psum_a if merge_tps else psum_t)
            if reps == 1:
                _emit_body(nc, pools, xr_d, x8_d, w8_d, tb_d, out_d, noload)
            else:
                with tc.For_i(0, reps, 1, staggered_reset=True,
                              hint_engines=(mybir.EngineType.PE,)):
                    _emit_body(nc, pools, xr_d, x8_d, w8_d, tb_d, out_d, noload)
    nc.compile()
    return nc


_NC_CACHE = {}


def _get_nc(reps=1):
    if reps not in _NC_CACHE:
        _NC_CACHE[reps] = build_nc(reps)
    return _NC_CACHE[reps]


def make_in_maps(x, diagonals):
    c = _circ_cols(diagonals)
    w8 = np.asarray(c[:, _E2R_IDX] * SCALE_W).astype(_np_f8)  # [S, 128, 4096]
    xT = np.asarray(x, dtype=np.float32).reshape(ROWS, L).T  # [L, ROWS] view
    tb = _TBAND.astype(np.float32)
    in_maps = []
    for i in range(NCORES):
        xs = np.ascontiguousarray(xT[:, i * RPC : (i + 1) * RPC])
        ch = xs.reshape(NCHUNK, 128, RPC)  # chunk-major
        xr = ch.transpose(1, 0, 2)  # [128, 16, RPC]
        xr = np.ascontiguousarray(xr.reshape(128, 4, 4, RPC).transpose(1, 0, 2, 3))
        x8 = np.ascontiguousarray(ch[_PAIR_IDX].transpose(1, 0, 2)).reshape(
            128, NPAIR, 2, RPC
        )
        in_maps.append(
            {
                "xr": xr.astype(np.float32),
                "x8": x8.astype(_np_f8),
                "w8": w8,
                "tb": tb,
            }
        )
    return in_maps


def gather_out(results):
    parts = []
    for r in results:
        o = r["out"]  # [128, NCHUNK, RPC]
        parts.append(np.ascontiguousarray(o.transpose(1, 0, 2)).reshape(L, RPC))
    outT = np.concatenate(parts, axis=1)  # [L, ROWS]
    return np.ascontiguousarray(outT.T).reshape(B, C, L).astype(np.float32)


def kernel(x, diagonals):
    x = np.asarray(x, dtype=np.float32)
    assert x.shape == (B, C, L) and np.asarray(diagonals).shape == (S, L)
    nc = _get_nc(1)
    in_maps = make_in_maps(x, diagonals)
    last_err = None
    for attempt in range(3):
        try:
            res = run_bass_kernel_spmd(nc, in_maps, core_ids=list(range(NCORES)))
            return gather_out(res.results)
        except Exception as ex:  # transient device errors (e.g. NRT_EXEC_UNIT_UNRECOVERABLE)
            last_err = ex
            import time as _time

            _time.sleep(2.0 * (attempt + 1))
    raise last_err


# revision 7
# speedup vs baseline: 1.4695x; 1.4695x over previous
"""Trainium2 Bass kernel for the seasonal-decomposition block.

Math: for each season s, circ_s = real(F_s^H diag(d_s) F_s) with F_s the s-th
diagonal LxL block of the normalized N=L*S DFT matrix. Expanding,
    circ_s[a, b] = (1/N) * sum_j d_s[j] * cos(2*pi*(s*L+j)*(a-b)/N)
depends only on a-b: a symmetric Toeplitz matrix whose first column
c_s(t) is computed on host with one length-N FFT. Every 128x128 block of
circ_s is a contiguous column slice of the skewed buffer
    E2r_s[p, m] = c_s(|2047 + p - m|)   (shape [128, 4096])
so the LxL matrix is never materialized.

The recurrence  x_rem <- x_rem - tanh(x_rem @ circ_s)  runs in transposed
layout (positions on partitions, rows on the free axis). The seasonal
matmuls run in fp8-e4m3 with MatmulPerfMode.DoubleRow: adjacent chunk
pairs (a=2t, 2t+1) occupy adjacent 128-column windows of the skewed
buffer, so a contiguous 256-column slice is a valid [128, 2, 128] lhsT
and one DoubleRow matmul retires two 128-deep accumulation steps at
2 rows/cycle. Weights are pre-scaled by 2^14 on host (max |c|*2^14 ~ 170
< 240) and the tanh activation descales with scale=2^-14. x_rem state
stays fp32 on-chip (fresh fp8 cast each season). Measured end-to-end
rel err ~1.2e-2 < 2e-2 tolerance.

Every matmul instruction is auto-paired with an LDWEIGHTS; for DoubleRow
that load is 256 columns (~184 ns) vs ~120 ns of matmul per 512 moving
rows, so single-chunk matmuls are weight-load-bound. The loop therefore
processes SAME-PARITY CHUNK PAIRS (b, b+2) into one [128, 512] PSUM bank:
a window's weight blocks depend only on u = b - 2t (Toeplitz), so one
DoubleRow matmul with rhs [128, 2, 512] (x8 stored plane-major, pair
slots adjacent) feeds both chunks and the weight load hides under the
doubled moving stream. Per group: 1 full-width start matmul, 2 edge
singles (pair 0 for chunk b+2, pair 7 for chunk b), 6 more doubles.
The pair-7 single sits 8th of 9 so the previous season's last tanh/sub/
cast lands before the PE needs it - no inter-season stall.

The trailing avg-pool trend runs in fp32r as two banded matmuls per
128-chunk; season 3 orders groups so trends interleave at group
boundaries. The output is accumulated as sum_s tanh_s + trend (never
x - x_rem, which would lose precision to cancellation).

Sharding: pure data-parallel over the B*C = 2048 rows, 256 rows per core,
8 cores, no collectives.
"""

import sys

sys.path.insert(0, "/opt/trn_rl_repo")

import ml_dtypes
import numpy as np

import concourse.mybir as mybir
import concourse.tile as tile
from concourse import bacc
from concourse.bass_utils import run_bass_kernel_spmd

L = 2048
S = 4
NFULL = L * S
KER = 25
B, C = 64, 32
NCORES = 8
ROWS = B * C          # 2048
RPC = ROWS // NCORES  # 256 rows per core
NCHUNK = L // 128     # 16
NPAIR = NCHUNK // 2   # 8

_f32 = mybir.dt.float32
_f32r = mybir.dt.float32r
_f8 = mybir.dt.float8e4
_np_f8 = ml_dtypes.float8_e4m3
SCALE_W = 2.0 ** 14

# group bases: same-parity chunk pairs (b, b+2)
_GROUPS = (0, 1, 4, 5, 8, 9, 12, 13)
# season 3: odd groups first (descending), then even groups descending so
# chunks complete in an order that lets trend chunks interleave.
_GROUPS_S3 = (13, 9, 5, 1, 12, 8, 4, 0)
_TREND_AFTER = {12: (15, 14, 13), 8: (12, 11, 10, 9), 4: (8, 7, 6, 5)}


def _build_tband():
    """Three [128,128] band blocks of the avg-pool matrix T (trend = T.T @ x)."""
    u = np.arange(128)[:, None]
    t = np.arange(128)[None, :]
    diag = ((t - u >= 0) & (t - u <= KER - 1)).astype(np.float32) / KER
    sub = ((u - t) >= 128 - (KER - 1)).astype(np.float32) / KER
    t00 = diag.copy()
    t00[0, :] += np.maximum(0, (KER - 1) - np.arange(128)).astype(np.float32) / KER
    return np.ascontiguousarray(np.stack([t00, diag, sub], axis=1))  # [128, 3, 128]


_TBAND = _build_tband()
_E2R_IDX = np.clip(np.abs(2047 + np.arange(128)[:, None] - np.arange(4096)[None, :]), 0, L - 1)


def _circ_cols(diagonals):
    """First columns c_s(t), t = 0..L-1, of each season's Toeplitz circ_s."""
    d = np.zeros((S, NFULL), dtype=np.float64)
    d[:, :L] = np.asarray(diagonals, dtype=np.float64)
    F = np.fft.fft(d, axis=1)  # F[s,k] = sum_j d_j e^{-2pi i jk/N}
    t = np.arange(L)
    ph = np.exp((2j * np.pi / NFULL) * (np.arange(S)[:, None] * L * t[None, :]))
    return ((ph * np.conj(F[:, :L])).real / NFULL).astype(np.float32)  # [S, L]


def _emit_body(nc, pools, xr_d, x8_d, w8_d, tb_d, out_d):
    constp, x8p, xrp, corrp, workp, psum_a, psum_t = pools
    tanh_f = mybir.ActivationFunctionType.Tanh
    dr = mybir.MatmulPerfMode.DoubleRow

    # Prologue DMA order follows first use: season-0 matmuls need x8 + w8[0];
    # the fp32 x quarters feed the DVE subs that start ~1us later.
    x8_t0 = x8p.tile([128, 2, NPAIR, RPC], _f8, tag="x8", name="x8_0")
    w8_sb = [constp.tile([128, 4096], _f8, tag=f"w8_{s}", name=f"w8_{s}") for s in range(S)]
    x0t = [constp.tile([128, 4, RPC], _f32r, tag=f"x0_{k}", name=f"x0_{k}") for k in range(4)]
    nc.sync.dma_start(x8_t0[:], x8_d[:])
    nc.sync.dma_start(w8_sb[0][:], w8_d[0])
    for k in range(4):
        nc.sync.dma_start(x0t[k][:], xr_d[k])
    for s in range(1, S):
        nc.sync.dma_start(w8_sb[s][:], w8_d[s])
    tb_sb = constp.tile([128, 3, 128], _f32r, tag="tb")
    nc.sync.dma_start(tb_sb[:], tb_d[:])

    xr_cur = [x0t[a // 4][:, a % 4, :] for a in range(NCHUNK)]
    corr = [corrp.tile([128, RPC], _f32, tag=f"corr{b}", name=f"corr{b}") for b in range(NCHUNK)]
    big_ob = constp.tile([128, NCHUNK, RPC], _f32, tag="bigob")

    def emit_trend(j, xr3):
        tps = psum_t.tile([128, RPC], _f32, tag="tps", name=f"tps{j}")
        if j == 0:
            nc.tensor.matmul(tps[:], tb_sb[:, 0, :], xr3[0], start=True, stop=True)
        else:
            nc.tensor.matmul(tps[:], tb_sb[:, 2, :], xr3[j - 1], start=True, stop=False)
            nc.tensor.matmul(tps[:], tb_sb[:, 1, :], xr3[j], start=False, stop=True)
        nc.vector.tensor_add(out=big_ob[:, j, :], in0=corr[j][:], in1=tps[:])
        if j % 4 == 0:
            q = j // 4
            nc.sync.dma_start(out_d[:, 4 * q : 4 * q + 4, :], big_ob[:, 4 * q : 4 * q + 4, :])

    def window(s, u):
        col0 = 1919 + 128 * u
        return w8_sb[s][:, col0 : col0 + 256].rearrange("p (two f) -> p two f", two=2)

    x8_cur = x8_t0
    for s in range(S):
        xr_next = [xrp.tile([128, RPC], _f32r, tag=f"xr{b}", name=f"xr{s}_{b}") for b in range(NCHUNK)]
        x8_next = (
            x8p.tile([128, 2, NPAIR, RPC], _f8, tag="x8", name=f"x8_{s + 1}")
            if s < S - 1
            else None
        )

        def finish_chunk(b, tmp_half):
            nc.vector.tensor_add(out=corr[b][:], in0=corr[b][:], in1=tmp_half)
            nc.vector.tensor_sub(out=xr_next[b][:], in0=xr_cur[b], in1=tmp_half)
            if s < S - 1:
                # fp8 cast for next season's rhs; plane0 = odd chunk, plane1 = even.
                nc.gpsimd.tensor_copy(
                    out=x8_next[:, 1 - (b % 2), b // 2, :], in_=xr_next[b][:]
                )

        groups = _GROUPS_S3 if s == S - 1 else _GROUPS
        for b in groups:
            acc = psum_a.tile([128, 2 * RPC], _f32, tag="acc", name=f"acc{s}_{b}")
            # 9 matmuls per group (b, b+2); pair-t double at window u = b - 2t
            # reads pair slots (t, t+1): chunk b gets block(1-u) x_{2t+1} +
            # block(-u) x_{2t}, chunk b+2 the same blocks on pairs shifted by
            # one - weight blocks depend only on u, shared by both chunks.
            nc.tensor.matmul(
                acc[:], window(s, b),
                x8_cur[:, :, 0:2, :].rearrange("p two g f -> p two (g f)"),
                start=True, stop=False, perf_mode=dr,
            )
            nc.tensor.matmul(
                acc[:, RPC:], window(s, b + 2), x8_cur[:, :, 0, :],
                start=False, stop=False, perf_mode=dr,
            )
            for t in range(1, 6):
                nc.tensor.matmul(
                    acc[:], window(s, b - 2 * t),
                    x8_cur[:, :, t : t + 2, :].rearrange("p two g f -> p two (g f)"),
                    start=False, stop=False, perf_mode=dr,
                )
            # pair-7 single sits here (8th of 9) so the previous season's last
            # cast (chunks 14/15) lands well before the PE reaches it.
            nc.tensor.matmul(
                acc[:, :RPC], window(s, b - 14), x8_cur[:, :, 7, :],
                start=False, stop=False, perf_mode=dr,
            )
            nc.tensor.matmul(
                acc[:], window(s, b - 12),
                x8_cur[:, :, 6:8, :].rearrange("p two g f -> p two (g f)"),
                start=False, stop=True, perf_mode=dr,
            )
            if s == 0:
                nc.scalar.activation(corr[b][:], acc[:, :RPC], tanh_f, scale=1.0 / SCALE_W)
                nc.scalar.activation(corr[b + 2][:], acc[:, RPC:], tanh_f, scale=1.0 / SCALE_W)
                for bb in (b, b + 2):
                    nc.vector.tensor_sub(out=xr_next[bb][:], in0=xr_cur[bb], in1=corr[bb][:])
                    if s < S - 1:
                        nc.gpsimd.tensor_copy(
                            out=x8_next[:, 1 - (bb % 2), bb // 2, :], in_=xr_next[bb][:]
                        )
            else:
                tmp = workp.tile([128, 2 * RPC], _f32, tag="tanh")
                nc.scalar.activation(tmp[:], acc[:], tanh_f, scale=1.0 / SCALE_W)
                finish_chunk(b, tmp[:, :RPC])
                finish_chunk(b + 2, tmp[:, RPC:])
            if s == S - 1 and b in _TREND_AFTER:
                for j in _TREND_AFTER[b]:
                    emit_trend(j, xr_next)
        xr_cur = [t_[:] for t_ in xr_next]
        x8_cur = x8_next

    for j in (4, 3, 2, 1, 0):
        emit_trend(j, xr_cur)


def build_nc(reps=1, acc_bufs=4):
    nc = bacc.Bacc("TRN2", target_bir_lowering=False, debug=False)
    xr_d = nc.dram_tensor("xr", [4, 128, 4, RPC], _f32r, kind="ExternalInput")
    x8_d = nc.dram_tensor("x8", [128, 2, NPAIR, RPC], _f8, kind="ExternalInput")
    w8_d = nc.dram_tensor("w8", [S, 128, 4096], _f8, kind="ExternalInput")
    tb_d = nc.dram_tensor("tb", [128, 3, 128], _f32r, kind="ExternalInput")
    out_d = nc.dram_tensor("out", [128, NCHUNK, RPC], _f32, kind="ExternalOutput")

    with tile.TileContext(nc) as tc:
        with (
            tc.tile_pool(name="const", bufs=1) as constp,
            tc.tile_pool(name="x8p", bufs=2) as x8p,
            tc.tile_pool(name="xrp", bufs=2) as xrp,
            tc.tile_pool(name="corrp", bufs=1) as corrp,
            tc.tile_pool(name="work", bufs=4) as workp,
            tc.tile_pool(name="psum_a", bufs=acc_bufs, space="PSUM") as psum_a,
            tc.tile_pool(name="psum_t", bufs=2, space="PSUM") as psum_t,
        ):
            pools = (constp, x8p, xrp, corrp, workp, psum_a, psum_t)
            if reps == 1:
                _emit_body(nc, pools, xr_d, x8_d, w8_d, tb_d, out_d)
            else:
                with tc.For_i(0, reps, 1, staggered_reset=True,
                              hint_engines=(mybir.EngineType.PE,)):
                    _emit_body(nc, pools, xr_d, x8_d, w8_d, tb_d, out_d)
    nc.compile()
    return nc


_NC_CACHE = {}


def _get_nc(reps=1):
    if reps not in _NC_CACHE:
        _NC_CACHE[reps] = build_nc(reps)
    return _NC_CACHE[reps]


def make_in_maps(x, diagonals):
    c = _circ_cols(diagonals)
    w8 = np.asarray(c[:, _E2R_IDX] * SCALE_W).astype(_np_f8)  # [S, 128, 4096]
    xT = np.asarray(x, dtype=np.float32).reshape(ROWS, L).T  # [L, ROWS] view
    tb = _TBAND.astype(np.float32)
    in_maps = []
    for i in range(NCORES):
        xs = np.ascontiguousarray(xT[:, i * RPC : (i + 1) * RPC])
        ch = xs.reshape(NCHUNK, 128, RPC)  # chunk-major
        xr = ch.transpose(1, 0, 2)  # [128, 16, RPC]
        xr = np.ascontiguousarray(xr.reshape(128, 4, 4, RPC).transpose(1, 0, 2, 3))
        # plane-major fp8: x8[p, 0, t, :] = chunk 2t+1, x8[p, 1, t, :] = chunk 2t
        x8 = np.stack([ch[1::2], ch[0::2]], axis=0)  # [2, 8, 128, RPC]
        x8 = np.ascontiguousarray(x8.transpose(2, 0, 1, 3))  # [128, 2, 8, RPC]
        in_maps.append(
            {
                "xr": xr.astype(np.float32),
                "x8": x8.astype(_np_f8),
                "w8": w8,
                "tb": tb,
            }
        )
    return in_maps


def gather_out(results):
    parts = []
    for r in results:
        o = r["out"]  # [128, NCHUNK, RPC]
        parts.append(np.ascontiguousarray(o.transpose(1, 0, 2)).reshape(L, RPC))
    outT = np.concatenate(parts, axis=1)  # [L, ROWS]
    return np.ascontiguousarray(outT.T).reshape(B, C, L).astype(np.float32)


def kernel(x, diagonals):
    x = np.asarray(x, dtype=np.float32)
    assert x.shape == (B, C, L) and np.asarray(diagonals).shape == (S, L)
    nc = _get_nc(1)
    in_maps = make_in_maps(x, diagonals)
    last_err = None
    for attempt in range(3):
        try:
            res = run_bass_kernel_spmd(nc, in_maps, core_ids=list(range(NCORES)))
            return gather_out(res.results)
        except Exception as ex:  # transient device errors (e.g. NRT_EXEC_UNIT_UNRECOVERABLE)
            last_err = ex
            import time as _time

            _time.sleep(2.0 * (attempt + 1))
    raise last_err
